# revision 20
# baseline (speedup 1.0000x reference)
"""Butterworth bandpass (cascaded biquad IIR) Trainium2 kernel.

Problem: y = sosfilt(sos, x) over x[32, 64, 4096] fp32 -- 2048 independent
signals, 4 cascaded DF2T biquads, sequential over T=4096.

Strategy (exact block reformulation, bf16 matmuls, no on-device transposes):
  The cascade is an 8-state linear system (A, B, C, D), balanced so all
  intermediates are O(1).  Host feeds each core x^T [T, 256] in bf16 (the
  transpose/cast is free host-side preprocessing) and reads back y^T, so
  every matmul operates in [time, signal] layout:

    per window of R=2 blocks of L=128 steps (s = window entry state):
      yT_r = Th @ xt_r + sum_{d<=r} (Z A_L^{d-1} F) @ xt_{r-d} + (Z A_L^r) @ s
      s'   = A_L^R @ s + sum_r (A_L^{R-1-r} F) @ xt_r

  All stationary operands (lhsT) are constant tables; x / s stream as rhs.
  PSUM accumulates fp32; results round to bf16 on the PSUM->SBUF copy and
  the host upcasts after gathering.

  Performance structure (measured on HW via perfetto traces):
  - The PE runs at a mid DVFS state (0.83 ns/row) until it has executed
    ~5us of *continuous, semaphore-wait-free* work, then doubles to 0.417
    ns/row and holds that through later waits.  NJUNK wait-free warm-up
    matmuls on a memset tile trigger the ramp while the input DMA streams
    in (values must be normal numbers -- garbage SBUF measurably slows
    the PE).
  - The whole input (512 KB bf16/core) lands in SBUF up front in 4 chunk
    transfers (SP queue; tables on the Act queue), so compute-phase
    matmuls carry almost no DMA waits.
  - Windows are processed in fused PAIRS: the host interleaves x blocks
    so conv/correction/state-update matmuls stream N=512 (two same-role
    blocks per instruction, halving instruction count and LDWEIGHTS
    exposure -- the toolchain reloads stationary weights per matmul).
    The two ZA (state->output) matmuls also fuse across the pair via an
    [s_w | s_{w+1}] rhs tile.  Only the tiny A_L^R state hops stay
    per-window (N=256); their serial PSUM->SBUF copies ride a dedicated
    DVE lane and are overlapped by neighbouring-pair matmuls.
  - y copies (PSUM -> bf16 stage) run on Act; 2-block output DMAs issue
    from the otherwise-idle SP queue as each piece completes.
"""

import numpy as np
import ml_dtypes

import concourse.bass as bass
import concourse.tile as tile
from concourse import bacc
from concourse import mybir
from concourse.bass_utils import run_bass_kernel_spmd

FP32 = mybir.dt.float32
BF16 = mybir.dt.bfloat16

L = 128            # time-block length (matmul contraction dim)
T = 4096
NCORES = 8
NSIG = 2048
SPC = NSIG // NCORES   # 256 signals per core
NST = 8            # state dim of the cascade
R = 2              # blocks per window
W = L * R
NW = T // W
NBLK = T // L          # 32 blocks
CHUNK = 8              # blocks per DMA chunk (4 windows)
NCHUNK = NBLK // CHUNK
OUTCHUNK = 4           # blocks per output DMA (2 windows)
NJUNK = 36             # warm-up matmuls (DVFS ramp) during the input phase


def _build_system(sos):
    sos = np.asarray(sos, dtype=np.float64)
    A = np.zeros((0, 0))
    B = np.zeros((0,))
    C = np.zeros((0,))
    D = 1.0
    for (b0, b1, b2, _one, a1, a2) in sos:
        As = np.array([[-a1, 1.0], [-a2, 0.0]])
        Bs = np.array([b1 - a1 * b0, b2 - a2 * b0])
        Cs = np.array([1.0, 0.0])
        Ds = b0
        n = A.shape[0]
        Anew = np.zeros((n + 2, n + 2))
        Anew[:n, :n] = A
        Anew[n:, :n] = np.outer(Bs, C)
        Anew[n:, n:] = As
        A = Anew
        B = np.concatenate([B, Bs * D])
        C = np.concatenate([Ds * C, Cs])
        D = Ds * D
    return A, B, C, D


def _balance(A, B, C):
    P = np.outer(B, B)
    Ak = A.copy()
    for _ in range(64):
        P = P + Ak @ P @ Ak.T
        Ak = Ak @ Ak
    Q = np.outer(C, C)
    Ak = A.copy()
    for _ in range(64):
        Q = Q + Ak.T @ Q @ Ak
        Ak = Ak @ Ak
    Rc = np.linalg.cholesky(P + 1e-30 * np.eye(len(B)))
    M = Rc.T @ Q @ Rc
    lam, U = np.linalg.eigh(M)
    lam = np.maximum(lam, 1e-30)
    Tm = Rc @ U @ np.diag(lam ** -0.25)
    Ti = np.diag(lam ** 0.25) @ U.T @ np.linalg.inv(Rc)
    return Ti @ A @ Tm, Ti @ B, C @ Tm


def _build_tables(sos):
    """lhsT tables (see module docstring), bf16.

    tabK [128, R*128 + R*8]: [Th | C_1.. | FT_0..]
    tab8 [8, R*128 + 8 + SPC]: [ZA_0.. | A2R | zeros (initial state)]
    """
    A, B, C, D = _build_system(sos)
    A, B, C = _balance(A, B, C)
    ns = A.shape[0]
    assert ns == NST

    h = np.zeros(L)
    h[0] = D
    An = np.eye(ns)
    for k in range(1, L):
        h[k] = C @ An @ B
        An = An @ A
    Th = np.zeros((L, L))
    for m in range(L):
        Th[m, m:] = h[: L - m]
    Z = np.zeros((L, ns))
    CAn = C.copy()
    for n in range(L):
        Z[n] = CAn
        CAn = CAn @ A
    F = np.zeros((ns, L))
    AmB = B.copy()
    for m in range(L - 1, -1, -1):
        F[:, m] = AmB
        AmB = A @ AmB
    AL = np.linalg.matrix_power(A, L)

    tabK = np.zeros((L, R * L + R * NST))
    tabK[:, :L] = Th
    for d in range(1, R):
        tabK[:, d * L:(d + 1) * L] = (Z @ np.linalg.matrix_power(AL, d - 1) @ F).T
    for r in range(R):
        tabK[:, R * L + r * NST:R * L + (r + 1) * NST] = (
            np.linalg.matrix_power(AL, R - 1 - r) @ F
        ).T
    tab8 = np.zeros((NST, R * L + NST + SPC))
    for r in range(R):
        tab8[:, r * L:(r + 1) * L] = (Z @ np.linalg.matrix_power(AL, r)).T
    tab8[:, R * L:R * L + NST] = np.linalg.matrix_power(AL, R).T

    b16 = lambda a: np.ascontiguousarray(a.astype(ml_dtypes.bfloat16))
    return b16(tabK), b16(tab8)


def _build_nc():
    nc = bacc.Bacc("TRN2", target_bir_lowering=False)
    # xt is host-reordered: per pair p of windows, blocks [4p, 4p+2, 4p+1,
    # 4p+3] so each fused N=512 matmul reads two same-r blocks adjacently
    xt_d = nc.dram_tensor("xt", [T, SPC], BF16, kind="ExternalInput").ap()
    tabk_d = nc.dram_tensor("tabk", [L, R * L + R * NST], BF16,
                            kind="ExternalInput").ap()
    tab8_d = nc.dram_tensor("tab8", [NST, R * L + NST + SPC], BF16,
                            kind="ExternalInput").ap()
    yt_d = nc.dram_tensor("yt", [T, SPC], BF16, kind="ExternalOutput").ap()

    NPAIR = NW // 2
    with tile.TileContext(nc) as tc:
        with (
            tc.tile_pool(name="consts", bufs=1) as consts,
            tc.tile_pool(name="xchunks", bufs=1) as xchunks,
            tc.tile_pool(name="ystage", bufs=1) as ystage,
            tc.tile_pool(name="spool", bufs=3) as spool,
            tc.tile_pool(name="pyp", bufs=3, space="PSUM") as pyp,
            tc.tile_pool(name="psp", bufs=2, space="PSUM") as psp,
        ):
            # warm-up tile via DVE memset: the junk matmuls below only warm
            # the PE's DVFS clock (values irrelevant, but must be normal
            # numbers: uninitialized garbage measurably slows the PE)
            warm = consts.tile([L, SPC], BF16, name="warm")
            nc.vector.memset(warm, 0.25)

            # x chunks on the SP queue; tables on the Act queue
            xc = []
            for c in range(NCHUNK):
                t = xchunks.tile([L, CHUNK * SPC], BF16, name=f"xc{c}")
                src = xt_d[c * CHUNK * L:(c + 1) * CHUNK * L, :].rearrange(
                    "(b p) s -> p b s", p=L)
                dst = t.rearrange("p (b s) -> p b s", b=CHUNK)
                nc.sync.dma_start(dst, src)
                xc.append(t)
            tab8_sb = consts.tile([NST, R * L + NST + SPC], BF16)
            nc.scalar.dma_start(tab8_sb, tab8_d)
            tabk_sb = consts.tile([L, R * L + R * NST], BF16)
            nc.scalar.dma_start(tabk_sb, tabk_d)

            def xop(p, r):
                """Fused rhs [128, 512] = blocks (4p+r, 4p+2+r) of pair p
                (adjacent thanks to the host reorder: slot 4p + 2*r)."""
                c, i = divmod(4 * p + 2 * r, CHUNK)
                return xc[c][:, i * SPC:(i + 2) * SPC]

            th_sb = tabk_sb[:, 0:L]
            c1_sb = tabk_sb[:, L:2 * L]
            ft_sb = [tabk_sb[:, R * L + r * NST:R * L + (r + 1) * NST]
                     for r in range(R)]
            za_sb = [tab8_sb[:, r * L:(r + 1) * L] for r in range(R)]
            a2r_sb = tab8_sb[:, R * L:R * L + NST]
            s_prev = tab8_sb[:, R * L + NST:]   # zeros: initial state

            yst = [ystage.tile([L, CHUNK * SPC], BF16, name=f"yst{c}")
                   for c in range(NCHUNK)]

            # wait-free warm-up matmuls on the memset tile (DVFS ramp)
            for j in range(NJUNK):
                junk = pyp.tile([L, 2 * SPC], FP32, tag="y0", name="junk")
                nc.tensor.matmul(junk[:, 0:SPC], warm[:, 0:L], warm,
                                 start=True, stop=True)

            pending = []

            def drain_one(alt=False):
                # alt=True routes to the secondary engines (epilogue)
                if not pending:
                    return
                kind, args = pending.pop(0)
                if kind == "copy":
                    seg, src_ap = args
                    if alt:
                        nc.vector.tensor_copy(seg, src_ap)
                    else:
                        nc.scalar.copy(seg, src_ap)  # y-copies on Act
                else:
                    c, i0 = args
                    ob0 = c * CHUNK + i0
                    dst = yt_d[ob0 * L:(ob0 + 2) * L, :].rearrange(
                        "(b p) s -> p b s", p=L)
                    s2 = yst[c][:, i0 * SPC:(i0 + 2) * SPC].rearrange(
                        "p (b s) -> p b s", b=2)
                    nc.sync.dma_start(dst, s2)

            NPAIR = NW // 2
            yp_of = {}
            ps_of = {}
            s2_of = {}

            def emit_x(p):
                yp = [pyp.tile([L, 2 * SPC], FP32, tag=f"y{r}",
                               name=f"yp{r}") for r in range(R)]
                ps = psp.tile([NST, 2 * SPC], FP32, tag="ps")
                yp_of[p] = yp
                ps_of[p] = ps
                nc.tensor.matmul(yp[0], th_sb, xop(p, 0),
                                 start=True, stop=False)
                nc.tensor.matmul(yp[1], th_sb, xop(p, 1),
                                 start=True, stop=False)
                nc.tensor.matmul(yp[1], c1_sb, xop(p, 0),
                                 start=False, stop=False)
                nc.tensor.matmul(ps, ft_sb[0], xop(p, 0),
                                 start=True, stop=False)
                nc.tensor.matmul(ps, ft_sb[1], xop(p, 1),
                                 start=False, stop=False)

            def s2_tile(p):
                if p not in s2_of:
                    s2_of[p] = spool.tile([NST, 2 * SPC], BF16, tag="s",
                                          name="s2")
                return s2_of[p]

            def emit_za(p, halves=(0, 1)):
                """Fused state->y matmuls for pair p (N=512 when both halves,
                one weight load each); stops yp(p) and queues its copies."""
                yp = yp_of[p]
                full = halves == (0, 1)
                for r in range(R):
                    if full:
                        nc.tensor.matmul(yp[r], za_sb[r], s2_of[p],
                                         start=False, stop=True,
                                         skip_group_check=True)
                c, i = divmod(4 * p, CHUNK)
                for h in halves:
                    col = slice(h * SPC, (h + 1) * SPC)
                    if not full:
                        for r in range(R):
                            nc.tensor.matmul(yp[r][:, col], za_sb[r],
                                             s2_of[p][:, col],
                                             start=False, stop=True,
                                             skip_group_check=True)
                    for r in range(R):
                        slot = i + 2 * h + r
                        seg = yst[c][:, slot * SPC:(slot + 1) * SPC]
                        pending.append(("copy", (seg, yp[r][:, col])))
                    pending.append(("dma", (c, i + 2 * h)))

            def emit_a2r(p, h):
                """State propagation for window half h of pair p; the DVE hop
                copy lands the result next to its sibling for the fused ZA."""
                ps = ps_of[p]
                col = slice(h * SPC, (h + 1) * SPC)
                nc.tensor.matmul(ps[:, col], a2r_sb, s2_of[p][:, col],
                                 start=False, stop=True,
                                 skip_group_check=True)
                k = 2 * p + h
                if k < NW - 1:
                    # state after window k -> slot (k+1): pair (k+1)//2,
                    # half (k+1)%2
                    pn, hn = divmod(k + 1, 2)
                    tgt = s2_tile(pn)
                    nc.vector.tensor_copy(
                        tgt[:, hn * SPC:(hn + 1) * SPC], ps[:, col])

            # prologue: initial state (zeros) + first pair's x matmuls
            nc.vector.memset(s2_tile(0)[:, 0:SPC], 0.0)
            emit_x(0)
            for p in range(NPAIR):
                if p >= 1:
                    emit_za(p - 1)
                emit_a2r(p, 0)
                drain_one()
                drain_one()
                drain_one()
                if p + 1 < NPAIR:
                    emit_x(p + 1)
                if p < NPAIR - 1:
                    emit_a2r(p, 1)
                else:
                    emit_za(p, halves=(0,))
                drain_one()
                drain_one()
                drain_one()
            emit_za(NPAIR - 1, halves=(1,))
            alt = False
            while pending:
                drain_one(alt)
                alt = not alt
    nc.compile()
    return nc


_NC_CACHE = None
LAST_RESULTS = None


def _get_nc():
    global _NC_CACHE
    if _NC_CACHE is None:
        _NC_CACHE = _build_nc()
    return _NC_CACHE


def kernel(x: np.ndarray, sos: np.ndarray) -> np.ndarray:
    x = np.asarray(x)
    orig_shape = x.shape
    orig_dtype = x.dtype
    tabk, tab8 = _build_tables(np.asarray(sos, dtype=np.float64))

    xt = x.reshape(NSIG, T).T.astype(ml_dtypes.bfloat16)   # [T, NSIG]
    # pair-interleave blocks: per pair p the device reads fused operands
    # [blk 4p | blk 4p+2] and [blk 4p+1 | blk 4p+3]
    xb = xt.reshape(NBLK, L, NSIG)
    order = []
    for p in range(NBLK // 4):
        order += [4 * p, 4 * p + 2, 4 * p + 1, 4 * p + 3]
    xt = np.ascontiguousarray(xb[order].reshape(T, NSIG))
    in_maps = [
        {"xt": np.ascontiguousarray(xt[:, c * SPC:(c + 1) * SPC]),
         "tabk": tabk, "tab8": tab8}
        for c in range(NCORES)
    ]
    nc = _get_nc()
    res = run_bass_kernel_spmd(nc, in_maps, core_ids=list(range(NCORES)))
    global LAST_RESULTS
    LAST_RESULTS = res
    yt = np.concatenate(
        [res.results[c]["yt"].astype(np.float32) for c in range(NCORES)],
        axis=1)
    return np.ascontiguousarray(yt.T).reshape(orig_shape).astype(
        orig_dtype, copy=False)
